# revision 18
# baseline (speedup 1.0000x reference)
"""Trainium2 Bass kernel for nn_AttentionBlock (GroupNorm + MHSA + proj + residual).

Sharding: 16 (batch, head) units over 8 cores -> 2 heads/core.
Cores 0-3: batch 0 head-pairs (0,1),(2,3),(4,5),(6,7); cores 4-7: batch 1.
Each core: GroupNorm its batch's x, compute q/k/v for its 2 heads, flash
attention (S^T layout), partial projection h = proj_w[:, its a-channels] @ a.
Core 0 of each batch also adds (xn + proj_b).  Host sums the 4 partials/batch.
"""

import sys

sys.path.insert(0, "/opt/trn_rl_repo")

import numpy as np

import concourse.bass as bass
import concourse.tile as tile
from concourse import mybir

F32 = mybir.dt.float32
F32R = mybir.dt.float32r
BF16 = mybir.dt.bfloat16
C = 512
D = 64
G = 32
EPS = 1e-6
SCALE = float(1.0 / np.sqrt(np.sqrt(D)))  # applied to q and k each
NCORES = 8


def _hoist_matmul_waits(nc):
    """This walrus build cannot encode sync waits on (self-loading) Matmult
    instructions ("Too many sync wait commands" in setupSyncWait<S3_LW>).
    Hoist every matmul's on_wait onto a PE no-op placed directly before it —
    same-engine in-order execution preserves semantics exactly."""
    k = 0
    for f in nc.m.functions:
        for bb in f.blocks:
            il = bb.instructions
            out = []
            changed = False
            for i in il:
                si = i.sync_info
                cls = i.__class__.__name__
                keep = 0 if cls in ("InstMatmult", "InstLdweights") else 1
                if (
                    cls != "InstNoOp"
                    and si is not None
                    and len(si.on_wait) > keep
                ):
                    waits = list(si.on_wait)
                    for w in waits[keep:]:
                        nop = mybir.InstNoOp(name=f"WH-{k}", ins=[], outs=[])
                        k += 1
                        nop.engine = i.engine
                        nop.sync_info = mybir.SyncInfo(on_wait=[w], on_update=[])
                        out.append(nop)
                    i.sync_info = mybir.SyncInfo(
                        on_wait=waits[:keep], on_update=list(si.on_update)
                    )
                    changed = True
                out.append(i)
            if changed:
                il[:] = out


def build_nc(n: int, hoist: bool = True):
    """Build the (SPMD-uniform) Bass kernel for sequence length n."""
    nc = bass.Bass()

    def mmr(out, lhsT, rhs, **kw):
        # float32r: single-pass fp32 matmul (vs 2-pass 2-cyc/elem full fp32)
        nc.tensor.matmul(out, lhsT.bitcast(F32R), rhs.bitcast(F32R), **kw)

    NI = n // 512  # i-blocks (queries)
    NJ = n // 128  # j-tiles (keys)
    NSG = max(1, n // 512)  # bn_stats subgroups (FMAX=512)

    x_d = nc.dram_tensor("x", [C, n], F32, kind="ExternalInput")
    wq_d = nc.dram_tensor("wq", [4, 128, 128], F32, kind="ExternalInput")
    wk_d = nc.dram_tensor("wk", [4, 128, 128], F32, kind="ExternalInput")
    wv_d = nc.dram_tensor("wv", [4, 128, 128], F32, kind="ExternalInput")
    qb_d = nc.dram_tensor("qb", [128, 1], F32, kind="ExternalInput")
    kb_d = nc.dram_tensor("kb", [128, 1], F32, kind="ExternalInput")
    vbb_d = nc.dram_tensor("vbb", [128, 128], F32, kind="ExternalInput")
    pct_d = nc.dram_tensor("pct", [128, 512], F32, kind="ExternalInput")
    pbr_d = nc.dram_tensor("pbr", [1, 512], F32, kind="ExternalInput")
    gnw_d = nc.dram_tensor("gnw", [4, 128, 1], F32, kind="ExternalInput")
    gnb_d = nc.dram_tensor("gnb", [4, 128, 1], F32, kind="ExternalInput")
    flag_d = nc.dram_tensor("flag", [128, 1], F32, kind="ExternalInput")
    bsel_d = nc.dram_tensor("bsel", [128, 128], F32, kind="ExternalInput")
    out_d = nc.dram_tensor("out", [C, n], F32, kind="ExternalOutput")

    AL = mybir.AluOpType
    AF = mybir.ActivationFunctionType

    with tile.TileContext(nc) as tc:
        with (
            tc.tile_pool(name="consts", bufs=1) as consts,
            tc.tile_pool(name="xp", bufs=1) as xp,
            tc.tile_pool(name="qkv", bufs=1) as qkv,
            tc.tile_pool(name="work", bufs=3) as work,
        ):
            # ---- load x (4 c-tiles, chunked so bn_stats overlaps the DMA) ----
            xs = []
            for ct in range(4):
                xt = xp.tile([128, n], F32, name=f"x{ct}", tag=f"x{ct}")
                for sg in range(NSG):
                    nc.sync.dma_start(
                        xt[:, sg * 512 : (sg + 1) * 512],
                        x_d[ct * 128 : (ct + 1) * 128, sg * 512 : (sg + 1) * 512],
                    )
                xs.append(xt)

            # ---- load constants ----
            wq_sb = consts.tile([128, 512], F32)
            wk_sb = consts.tile([128, 512], F32)
            wv_sb = consts.tile([128, 512], F32)
            for ct in range(4):
                nc.sync.dma_start(wq_sb[:, ct * 128 : (ct + 1) * 128], wq_d[ct])
                nc.sync.dma_start(wk_sb[:, ct * 128 : (ct + 1) * 128], wk_d[ct])
                nc.sync.dma_start(wv_sb[:, ct * 128 : (ct + 1) * 128], wv_d[ct])
            qb_sb = consts.tile([128, 1], F32)
            kb_sb = consts.tile([128, 1], F32)
            nc.sync.dma_start(qb_sb, qb_d[:])
            nc.sync.dma_start(kb_sb, kb_d[:])
            vbb_sb = consts.tile([128, 128], F32)
            nc.sync.dma_start(vbb_sb, vbb_d[:])
            pct_sb = consts.tile([128, 512], F32)
            nc.sync.dma_start(pct_sb, pct_d[:])
            pbr_sb = consts.tile([1, 512], F32)
            nc.sync.dma_start(pbr_sb, pbr_d[:])
            gnw_sb = consts.tile([128, 4], F32)
            gnb_sb = consts.tile([128, 4], F32)
            for ct in range(4):
                nc.sync.dma_start(gnw_sb[:, ct : ct + 1], gnw_d[ct])
                nc.sync.dma_start(gnb_sb[:, ct : ct + 1], gnb_d[ct])
            flag_sb = consts.tile([128, 1], F32)
            nc.sync.dma_start(flag_sb, flag_d[:])
            bsel_sb = consts.tile([128, 128], F32)
            nc.sync.dma_start(bsel_sb, bsel_d[:])
            eps_sb = consts.tile([128, 1], F32)
            nc.vector.memset(eps_sb, EPS)
            onescol = consts.tile([128, 64], BF16)
            nc.vector.memset(onescol, 0.0)
            nc.vector.memset(onescol[:, 0:1], 1.0)
            onesrow_f = consts.tile([1, 512], F32)
            nc.vector.memset(onesrow_f, 1.0)
            onesrow = consts.tile([1, 512], F32R)
            nc.vector.tensor_copy(onesrow, onesrow_f)
            # bf16 weight copies for the attention-path matmuls
            wqb = consts.tile([128, 512], BF16)
            wkb = consts.tile([128, 512], BF16)
            wvb = consts.tile([128, 512], BF16)
            nc.vector.tensor_copy(wqb, wq_sb)
            nc.vector.tensor_copy(wkb, wk_sb)
            nc.vector.tensor_copy(wvb, wv_sb)
            # f32r-rounded copies for the projection matmuls
            pct_r = consts.tile([128, 512], F32R)
            nc.vector.tensor_copy(pct_r, pct_sb)
            pbr_r = consts.tile([1, 512], F32R)
            nc.vector.tensor_copy(pbr_r, pbr_sb)

            q_sb = qkv.tile([128, n], BF16)
            k_sb = qkv.tile([128, n], BF16)
            vt_sb = qkv.tile([128, n], BF16)
            a_fin = qkv.tile([128, n], F32)
            xb = [qkv.tile([128, n], BF16, name=f"xb{ct}", tag=f"xb{ct}") for ct in range(4)]

            with tc.tile_pool(name="ppA", bufs=1, space="PSUM") as ppA:
                # ---- Phase A: GroupNorm, per c-tile (overlaps chunked DMA) ----
                for ct in range(4):
                    stats = work.tile([128, NSG, 6], F32, tag="stats")
                    for sg in range(NSG):
                        nc.vector.bn_stats(
                            stats[:, sg, :], xs[ct][:, sg * 512 : (sg + 1) * 512]
                        )
                    mv = work.tile([128, 2], F32, tag="mv")
                    nc.vector.bn_aggr(mv, stats)
                    e2 = work.tile([128, 2], F32, tag="e2")
                    nc.vector.tensor_copy(e2[:, 0:1], mv[:, 0:1])
                    nc.vector.tensor_tensor(e2[:, 1:2], mv[:, 0:1], mv[:, 0:1], AL.mult)
                    nc.vector.tensor_add(e2[:, 1:2], e2[:, 1:2], mv[:, 1:2])
                    gs_ps = ppA.tile([128, 2], F32, tag="gs")
                    nc.tensor.matmul(gs_ps, bsel_sb, e2, start=True, stop=True)
                    gsb = work.tile([128, 2], F32, tag="gsb")
                    nc.vector.tensor_copy(gsb, gs_ps)
                    var = work.tile([128, 1], F32, tag="var")
                    nc.vector.tensor_tensor(var, gsb[:, 0:1], gsb[:, 0:1], AL.mult)
                    nc.vector.tensor_sub(var, gsb[:, 1:2], var)
                    rstd = work.tile([128, 1], F32, tag="rstd")
                    nc.scalar.activation(rstd, var, AF.Sqrt, bias=eps_sb)
                    nc.vector.reciprocal(rstd, rstd)
                    ac = work.tile([128, 1], F32, tag="ac")
                    nc.vector.tensor_tensor(ac, rstd, gnw_sb[:, ct : ct + 1], AL.mult)
                    bc = work.tile([128, 1], F32, tag="bc")
                    nc.vector.tensor_tensor(bc, gsb[:, 0:1], ac, AL.mult)
                    nc.vector.tensor_sub(bc, gnb_sb[:, ct : ct + 1], bc)
                    # xn = x * ac + bc  (in place), then bf16 copy
                    nc.vector.tensor_scalar(
                        xs[ct], xs[ct], ac, bc, op0=AL.mult, op1=AL.add
                    )
                    nc.vector.tensor_copy(xb[ct], xs[ct])

                # ---- Phase B (minimal): k for all i-blocks, q for i-block 0 ----
                def emit_q(ib, pool, tag):
                    q_ps = pool.tile([128, 512], F32, tag=tag)
                    for ct in range(4):
                        nc.tensor.matmul(
                            q_ps,
                            wqb[:, ct * 128 : (ct + 1) * 128],
                            xb[ct][:, ib * 512 : (ib + 1) * 512],
                            start=(ct == 0), stop=(ct == 3),
                            skip_group_check=True,
                        )
                    nc.vector.tensor_scalar(
                        q_sb[:, ib * 512 : (ib + 1) * 512],
                        q_ps, qb_sb, SCALE, op0=AL.add, op1=AL.mult,
                    )

                def emit_vt(jt, pool, tag):
                    vt_ps = pool.tile([128, 128], F32, tag=tag)
                    for ct in range(4):
                        nc.tensor.matmul(
                            vt_ps,
                            xb[ct][:, jt * 128 : (jt + 1) * 128],
                            wvb[:, ct * 128 : (ct + 1) * 128],
                            start=(ct == 0), stop=(ct == 3),
                            skip_group_check=True,
                        )
                    nc.vector.tensor_tensor(
                        vt_sb[:, jt * 128 : (jt + 1) * 128], vt_ps, vbb_sb, AL.add
                    )

                for ib in range(NI):
                    k_ps = ppA.tile([128, 512], F32, tag="qk")
                    for ct in range(4):
                        nc.tensor.matmul(
                            k_ps,
                            wkb[:, ct * 128 : (ct + 1) * 128],
                            xb[ct][:, ib * 512 : (ib + 1) * 512],
                            start=(ct == 0), stop=(ct == 3),
                            skip_group_check=True,
                        )
                    nc.vector.tensor_scalar(
                        k_sb[:, ib * 512 : (ib + 1) * 512],
                        k_ps, kb_sb, SCALE, op0=AL.add, op1=AL.mult,
                    )
                emit_q(0, ppA, "qk")

            # ---- Phase C: flash attention + projection ----
            # Software-pipelined: av/rs lag the S^T+exp emission by one j-tile
            # (keeps the in-order PE queue from stalling on ACT); the
            # normalize chain for i-block P is emitted at jt==2 of P+1 and its
            # projection at jt==12; v^T tiles and q for i-blocks >= 1 are
            # emitted inside i-block 0's loop, hidden under the ACT-bound
            # steady state.
            with (
                tc.tile_pool(name="ppB", bufs=2, space="PSUM") as ppB,
                tc.tile_pool(name="dsc", bufs=3, space="DRAM") as dsc,
            ):

                def emit_norm(avp, rsp, islp):
                    rcp = work.tile([128, 512], F32, tag="rcp")
                    # rows 1-63 are matmul-written zeros -> inf, never read
                    nc.vector.reciprocal(rcp[0:65, :], rsp[0:65, :])
                    # broadcast rcp rows over partitions via a DRAM bounce
                    # (SBUF DMA reads cannot have partition step 0; DRAM can)
                    scr = dsc.tile([2, 512], F32, tag="scr")
                    nc.sync.dma_start(scr, rcp[0:128:64, :])
                    rcpb = work.tile([128, 512], F32, tag="rcpb")
                    nc.sync.dma_start(
                        rcpb[0:64, :], scr[0:1, :].broadcast_to((64, 512))
                    )
                    nc.sync.dma_start(
                        rcpb[64:128, :], scr[1:2, :].broadcast_to((64, 512))
                    )
                    nc.vector.tensor_tensor(
                        a_fin[:, islp].bitcast(F32R), avp, rcpb, AL.mult
                    )

                def emit_proj(islp):
                    for ot in range(4):
                        pp = ppB.tile([128, 512], F32, tag="av")
                        mmr(
                            pp, pct_r[:, ot * 128 : (ot + 1) * 128],
                            a_fin[:, islp], start=True, stop=False,
                            skip_group_check=True,
                        )
                        mmr(
                            pp, pbr_r[:, ot * 128 : (ot + 1) * 128],
                            onesrow, start=False, stop=True,
                            skip_group_check=True,
                        )
                        o_sb = work.tile([128, 512], F32, tag="osb")
                        nc.vector.scalar_tensor_tensor(
                            o_sb, xs[ot][:, islp], flag_sb, pp,
                            op0=AL.mult, op1=AL.add,
                        )
                        nc.sync.dma_start(
                            out_d[ot * 128 : (ot + 1) * 128, islp], o_sb
                        )

                jt_norm, jt_proj = 2, min(12, NJ - 1)
                VT_AHEAD = 3
                prev = None
                for ib in range(NI):
                    isl = slice(ib * 512, (ib + 1) * 512)
                    av = ppB.tile([128, 512], F32, tag="av")
                    rs = ppB.tile([128, 512], F32, tag="rs")

                    def emit_avrs(jt, e, av=av, rs=rs):
                        # a += v @ e ; col-packed heads at out partitions 0/64;
                        # rowsums via ones-column matmuls (same PE mode)
                        first, last = jt == 0, jt == NJ - 1
                        nc.tensor.matmul(
                            av[0:64, :], vt_sb[:, jt * 128 : jt * 128 + 64],
                            e[:, 0:512], start=first, stop=last,
                            skip_group_check=True,
                        )
                        nc.tensor.matmul(
                            av[64:128, :], vt_sb[:, jt * 128 + 64 : jt * 128 + 128],
                            e[:, 512:1024], start=first, stop=last,
                            skip_group_check=True,
                        )
                        nc.tensor.matmul(
                            rs[0:64, :], onescol, e[:, 0:512],
                            start=first, stop=last, skip_group_check=True,
                        )
                        nc.tensor.matmul(
                            rs[64:128, :], onescol, e[:, 512:1024],
                            start=first, stop=last, skip_group_check=True,
                        )

                    if ib == 0:
                        for jt in range(min(VT_AHEAD, NJ)):
                            emit_vt(jt, ppB, "av")

                    pend = []  # (jt, e): av/rs emission lags two jt
                    for jt in range(NJ):
                        jsl = slice(jt * 128, (jt + 1) * 128)
                        st = ppB.tile([128, 1024], F32, tag="st")
                        # S^T[j, i] for both heads (row-packed, K=64 each)
                        nc.tensor.matmul(
                            st[:, 0:512], k_sb[0:64, jsl], q_sb[0:64, isl],
                            start=True, stop=True, skip_group_check=True,
                        )
                        nc.tensor.matmul(
                            st[:, 512:1024], k_sb[64:128, jsl], q_sb[64:128, isl],
                            start=True, stop=True, skip_group_check=True,
                        )
                        e32 = work.tile([128, 1024], F32, tag="e32")
                        nc.scalar.activation(e32, st, AF.Exp)
                        e = work.tile([128, 1024], BF16, tag="esb")
                        nc.gpsimd.tensor_copy(e, e32)
                        if ib == 0:
                            if jt + VT_AHEAD < NJ:
                                emit_vt(jt + VT_AHEAD, ppB, "av")
                            if jt >= 8 and (jt - 8) % 3 == 0 and (iq := (jt - 8) // 3 + 1) < NI:
                                emit_q(iq, ppB, "av")
                        pend.append((jt, e))
                        if len(pend) > 2:
                            emit_avrs(*pend.pop(0))
                        if prev is not None:
                            if jt == jt_norm:
                                emit_norm(*prev)
                            if jt == jt_proj:
                                emit_proj(prev[2])
                                prev = None
                    for p_ in pend:
                        emit_avrs(*p_)
                    prev = (av, rs, isl)
                emit_norm(*prev)
                emit_proj(prev[2])
    if hoist:
        _hoist_matmul_waits(nc)
    return nc


def make_in_maps(x, gn_weight, gn_bias, qkv_w, qkv_b, proj_w, proj_b, n):
    """Per-core input dicts (pure slicing / transposition / constant setup)."""
    bsel = np.kron(np.eye(8, dtype=np.float32), np.full((16, 16), 1.0 / 16.0, np.float32))
    zeros_pb = np.zeros((1, 512), np.float32)
    in_maps = []
    for core in range(NCORES):
        bi, p = divmod(core, 4)
        h0, h1 = 2 * p, 2 * p + 1
        rq = np.concatenate([qkv_w[192 * h : 192 * h + 64] for h in (h0, h1)])
        rk = np.concatenate([qkv_w[192 * h + 64 : 192 * h + 128] for h in (h0, h1)])
        rv = np.concatenate([qkv_w[192 * h + 128 : 192 * h + 192] for h in (h0, h1)])
        bq = np.concatenate([qkv_b[192 * h : 192 * h + 64] for h in (h0, h1)])
        bk = np.concatenate([qkv_b[192 * h + 64 : 192 * h + 128] for h in (h0, h1)])
        bv = np.concatenate([qkv_b[192 * h + 128 : 192 * h + 192] for h in (h0, h1)])
        flag = 1.0 if p == 0 else 0.0
        in_maps.append({
            "x": np.ascontiguousarray(x[bi].reshape(C, n)),
            "wq": np.ascontiguousarray(rq.T.reshape(4, 128, 128)),
            "wk": np.ascontiguousarray(rk.T.reshape(4, 128, 128)),
            "wv": np.ascontiguousarray(rv.T.reshape(4, 128, 128)),
            "qb": np.ascontiguousarray(bq.reshape(128, 1)),
            "kb": np.ascontiguousarray(bk.reshape(128, 1)),
            "vbb": np.ascontiguousarray(np.broadcast_to(bv[None, :], (128, 128))),
            "pct": np.ascontiguousarray(proj_w[:, 128 * p : 128 * (p + 1)].T),
            "pbr": (proj_b.reshape(1, 512).astype(np.float32) if flag else zeros_pb),
            "gnw": np.ascontiguousarray(gn_weight.reshape(4, 128, 1)),
            "gnb": np.ascontiguousarray(gn_bias.reshape(4, 128, 1)),
            "flag": np.full((128, 1), flag, np.float32),
            "bsel": bsel,
        })
    return in_maps


_NC_CACHE = {}


def kernel(x, gn_weight, gn_bias, qkv_w, qkv_b, proj_w, proj_b):
    b, c, H, W = x.shape
    n = H * W
    if n not in _NC_CACHE:
        _NC_CACHE[n] = build_nc(n)
    nc = _NC_CACHE[n]
    in_maps = make_in_maps(
        np.asarray(x, np.float32), np.asarray(gn_weight, np.float32),
        np.asarray(gn_bias, np.float32), np.asarray(qkv_w, np.float32),
        np.asarray(qkv_b, np.float32), np.asarray(proj_w, np.float32),
        np.asarray(proj_b, np.float32), n,
    )
    from concourse.bass_utils import run_bass_kernel_spmd

    res = run_bass_kernel_spmd(nc, in_maps, core_ids=list(range(NCORES)))
    parts = [r["out"] for r in res.results]
    out = np.empty((b, c, n), np.float32)
    for bi in range(b):
        acc = parts[4 * bi].astype(np.float32)
        for j in range(1, 4):
            acc = acc + parts[4 * bi + j]
        out[bi] = acc
    return out.reshape(b, c, H, W)


# revision 19
# speedup vs baseline: 2.3985x; 2.3985x over previous
"""Trainium2 Bass kernel for nn_AttentionBlock (GroupNorm + MHSA + proj + residual).

Sharding: 16 (batch, head) units over 8 cores -> 2 heads/core.
Cores 0-3: batch 0 head-pairs (0,1),(2,3),(4,5),(6,7); cores 4-7: batch 1.
Each core: GroupNorm its batch's x, compute q/k/v for its 2 heads, flash
attention (S^T layout), partial projection h = proj_w[:, its a-channels] @ a.
Core 0 of each batch also adds (xn + proj_b).  Host sums the 4 partials/batch.
"""

import sys

sys.path.insert(0, "/opt/trn_rl_repo")

import numpy as np

import concourse.bass as bass
import concourse.tile as tile
from concourse import mybir

F32 = mybir.dt.float32
F32R = mybir.dt.float32r
BF16 = mybir.dt.bfloat16
C = 512
D = 64
G = 32
EPS = 1e-6
SCALE = float(1.0 / np.sqrt(np.sqrt(D)))  # applied to q and k each
NCORES = 8


def _hoist_matmul_waits(nc):
    """This walrus build cannot encode sync waits on (self-loading) Matmult
    instructions ("Too many sync wait commands" in setupSyncWait<S3_LW>).
    Hoist every matmul's on_wait onto a PE no-op placed directly before it —
    same-engine in-order execution preserves semantics exactly."""
    k = 0
    for f in nc.m.functions:
        for bb in f.blocks:
            il = bb.instructions
            out = []
            changed = False
            for i in il:
                si = i.sync_info
                cls = i.__class__.__name__
                keep = 0 if cls in ("InstMatmult", "InstLdweights") else 1
                if (
                    cls != "InstNoOp"
                    and si is not None
                    and len(si.on_wait) > keep
                ):
                    waits = list(si.on_wait)
                    for w in waits[keep:]:
                        nop = mybir.InstNoOp(name=f"WH-{k}", ins=[], outs=[])
                        k += 1
                        nop.engine = i.engine
                        nop.sync_info = mybir.SyncInfo(on_wait=[w], on_update=[])
                        out.append(nop)
                    i.sync_info = mybir.SyncInfo(
                        on_wait=waits[:keep], on_update=list(si.on_update)
                    )
                    changed = True
                out.append(i)
            if changed:
                il[:] = out


def build_nc(n: int, hoist: bool = True):
    """Build the (SPMD-uniform) Bass kernel for sequence length n."""
    nc = bass.Bass()

    def mmr(out, lhsT, rhs, **kw):
        # float32r: single-pass fp32 matmul (vs 2-pass 2-cyc/elem full fp32)
        nc.tensor.matmul(out, lhsT.bitcast(F32R), rhs.bitcast(F32R), **kw)

    NI = n // 512  # i-blocks (queries)
    NJ = n // 128  # j-tiles (keys)
    NSG = max(1, n // 512)  # bn_stats subgroups (FMAX=512)

    x_d = nc.dram_tensor("x", [C, n], F32, kind="ExternalInput")
    wq_d = nc.dram_tensor("wq", [4, 128, 128], F32, kind="ExternalInput")
    wk_d = nc.dram_tensor("wk", [4, 128, 128], F32, kind="ExternalInput")
    wv_d = nc.dram_tensor("wv", [4, 128, 128], F32, kind="ExternalInput")
    qb_d = nc.dram_tensor("qb", [128, 1], F32, kind="ExternalInput")
    kb_d = nc.dram_tensor("kb", [128, 1], F32, kind="ExternalInput")
    vbb_d = nc.dram_tensor("vbb", [128, 128], F32, kind="ExternalInput")
    pct_d = nc.dram_tensor("pct", [128, 512], F32, kind="ExternalInput")
    pbr_d = nc.dram_tensor("pbr", [1, 512], F32, kind="ExternalInput")
    gnw_d = nc.dram_tensor("gnw", [4, 128, 1], F32, kind="ExternalInput")
    gnb_d = nc.dram_tensor("gnb", [4, 128, 1], F32, kind="ExternalInput")
    flag_d = nc.dram_tensor("flag", [128, 1], F32, kind="ExternalInput")
    bsel_d = nc.dram_tensor("bsel", [128, 128], F32, kind="ExternalInput")
    out_d = nc.dram_tensor("out", [C, n], F32, kind="ExternalOutput")

    AL = mybir.AluOpType
    AF = mybir.ActivationFunctionType

    with tile.TileContext(nc) as tc:
        with (
            tc.tile_pool(name="consts", bufs=1) as consts,
            tc.tile_pool(name="xp", bufs=1) as xp,
            tc.tile_pool(name="qkv", bufs=1) as qkv,
            tc.tile_pool(name="work", bufs=3) as work,
        ):
            # ---- load x (4 c-tiles, chunked so bn_stats overlaps the DMA) ----
            xs = []
            for ct in range(4):
                xt = xp.tile([128, n], F32, name=f"x{ct}", tag=f"x{ct}")
                for sg in range(NSG):
                    nc.sync.dma_start(
                        xt[:, sg * 512 : (sg + 1) * 512],
                        x_d[ct * 128 : (ct + 1) * 128, sg * 512 : (sg + 1) * 512],
                    )
                xs.append(xt)

            # ---- load constants ----
            wq_sb = consts.tile([128, 512], F32)
            wk_sb = consts.tile([128, 512], F32)
            wv_sb = consts.tile([128, 512], F32)
            for ct in range(4):
                nc.sync.dma_start(wq_sb[:, ct * 128 : (ct + 1) * 128], wq_d[ct])
                nc.sync.dma_start(wk_sb[:, ct * 128 : (ct + 1) * 128], wk_d[ct])
                nc.sync.dma_start(wv_sb[:, ct * 128 : (ct + 1) * 128], wv_d[ct])
            qb_sb = consts.tile([128, 1], F32)
            kb_sb = consts.tile([128, 1], F32)
            nc.sync.dma_start(qb_sb, qb_d[:])
            nc.sync.dma_start(kb_sb, kb_d[:])
            vbb_sb = consts.tile([128, 128], F32)
            nc.sync.dma_start(vbb_sb, vbb_d[:])
            pct_sb = consts.tile([128, 512], F32)
            nc.sync.dma_start(pct_sb, pct_d[:])
            pbr_sb = consts.tile([1, 512], F32)
            nc.sync.dma_start(pbr_sb, pbr_d[:])
            gnw_sb = consts.tile([128, 4], F32)
            gnb_sb = consts.tile([128, 4], F32)
            for ct in range(4):
                nc.sync.dma_start(gnw_sb[:, ct : ct + 1], gnw_d[ct])
                nc.sync.dma_start(gnb_sb[:, ct : ct + 1], gnb_d[ct])
            flag_sb = consts.tile([128, 1], F32)
            nc.sync.dma_start(flag_sb, flag_d[:])
            bsel_sb = consts.tile([128, 128], F32)
            nc.sync.dma_start(bsel_sb, bsel_d[:])
            eps_sb = consts.tile([128, 1], F32)
            nc.vector.memset(eps_sb, EPS)
            onescol = consts.tile([128, 64], BF16)
            nc.vector.memset(onescol, 0.0)
            nc.vector.memset(onescol[:, 0:1], 1.0)
            onesrow_f = consts.tile([1, 512], F32)
            nc.vector.memset(onesrow_f, 1.0)
            onesrow = consts.tile([1, 512], F32R)
            nc.vector.tensor_copy(onesrow, onesrow_f)
            # bf16 weight copies for the attention-path matmuls
            wqb = consts.tile([128, 512], BF16)
            wkb = consts.tile([128, 512], BF16)
            wvb = consts.tile([128, 512], BF16)
            nc.vector.tensor_copy(wqb, wq_sb)
            nc.vector.tensor_copy(wkb, wk_sb)
            nc.vector.tensor_copy(wvb, wv_sb)
            # f32r-rounded copies for the projection matmuls
            pct_r = consts.tile([128, 512], F32R)
            nc.vector.tensor_copy(pct_r, pct_sb)
            pbr_r = consts.tile([1, 512], F32R)
            nc.vector.tensor_copy(pbr_r, pbr_sb)

            q_sb = qkv.tile([128, n], BF16)
            k_sb = qkv.tile([128, n], BF16)
            vt_sb = qkv.tile([128, n], BF16)
            a_fin = qkv.tile([128, n], F32)
            xb = [qkv.tile([128, n], BF16, name=f"xb{ct}", tag=f"xb{ct}") for ct in range(4)]

            with tc.tile_pool(name="ppA", bufs=1, space="PSUM") as ppA:
                # ---- Phase A: GroupNorm, per c-tile (overlaps chunked DMA) ----
                for ct in range(4):
                    stats = work.tile([128, NSG, 6], F32, tag="stats")
                    for sg in range(NSG):
                        nc.vector.bn_stats(
                            stats[:, sg, :], xs[ct][:, sg * 512 : (sg + 1) * 512]
                        )
                    mv = work.tile([128, 2], F32, tag="mv")
                    nc.vector.bn_aggr(mv, stats)
                    e2 = work.tile([128, 2], F32, tag="e2")
                    nc.vector.tensor_copy(e2[:, 0:1], mv[:, 0:1])
                    nc.vector.tensor_tensor(e2[:, 1:2], mv[:, 0:1], mv[:, 0:1], AL.mult)
                    nc.vector.tensor_add(e2[:, 1:2], e2[:, 1:2], mv[:, 1:2])
                    gs_ps = ppA.tile([128, 2], F32, tag="gs")
                    nc.tensor.matmul(gs_ps, bsel_sb, e2, start=True, stop=True)
                    gsb = work.tile([128, 2], F32, tag="gsb")
                    nc.vector.tensor_copy(gsb, gs_ps)
                    var = work.tile([128, 1], F32, tag="var")
                    nc.vector.tensor_tensor(var, gsb[:, 0:1], gsb[:, 0:1], AL.mult)
                    nc.vector.tensor_sub(var, gsb[:, 1:2], var)
                    rstd = work.tile([128, 1], F32, tag="rstd")
                    nc.scalar.activation(rstd, var, AF.Sqrt, bias=eps_sb)
                    nc.vector.reciprocal(rstd, rstd)
                    ac = work.tile([128, 1], F32, tag="ac")
                    nc.vector.tensor_tensor(ac, rstd, gnw_sb[:, ct : ct + 1], AL.mult)
                    bc = work.tile([128, 1], F32, tag="bc")
                    nc.vector.tensor_tensor(bc, gsb[:, 0:1], ac, AL.mult)
                    nc.vector.tensor_sub(bc, gnb_sb[:, ct : ct + 1], bc)
                    # xn = x * ac + bc  (in place), then bf16 copy
                    nc.vector.tensor_scalar(
                        xs[ct], xs[ct], ac, bc, op0=AL.mult, op1=AL.add
                    )
                    nc.vector.tensor_copy(xb[ct], xs[ct])

                # ---- Phase B (minimal): k for all i-blocks, q for i-block 0 ----
                def emit_q(ib, pool, tag):
                    q_ps = pool.tile([128, 512], F32, tag=tag)
                    for ct in range(4):
                        nc.tensor.matmul(
                            q_ps,
                            wqb[:, ct * 128 : (ct + 1) * 128],
                            xb[ct][:, ib * 512 : (ib + 1) * 512],
                            start=(ct == 0), stop=(ct == 3),
                            skip_group_check=True,
                        )
                    nc.vector.tensor_scalar(
                        q_sb[:, ib * 512 : (ib + 1) * 512],
                        q_ps, qb_sb, SCALE, op0=AL.add, op1=AL.mult,
                    )

                def emit_vt(jt, pool, tag):
                    vt_ps = pool.tile([128, 128], F32, tag=tag)
                    for ct in range(4):
                        nc.tensor.matmul(
                            vt_ps,
                            xb[ct][:, jt * 128 : (jt + 1) * 128],
                            wvb[:, ct * 128 : (ct + 1) * 128],
                            start=(ct == 0), stop=(ct == 3),
                            skip_group_check=True,
                        )
                    nc.vector.tensor_tensor(
                        vt_sb[:, jt * 128 : (jt + 1) * 128], vt_ps, vbb_sb, AL.add
                    )

                for ib in range(NI):
                    k_ps = ppA.tile([128, 512], F32, tag="qk")
                    for ct in range(4):
                        nc.tensor.matmul(
                            k_ps,
                            wkb[:, ct * 128 : (ct + 1) * 128],
                            xb[ct][:, ib * 512 : (ib + 1) * 512],
                            start=(ct == 0), stop=(ct == 3),
                            skip_group_check=True,
                        )
                    nc.vector.tensor_scalar(
                        k_sb[:, ib * 512 : (ib + 1) * 512],
                        k_ps, kb_sb, SCALE, op0=AL.add, op1=AL.mult,
                    )
                emit_q(0, ppA, "qk")

            # ---- Phase C: flash attention + projection ----
            # Software-pipelined: av/rs lag the S^T+exp emission by one j-tile
            # (keeps the in-order PE queue from stalling on ACT); the
            # normalize chain for i-block P is emitted at jt==2 of P+1 and its
            # projection at jt==12; v^T tiles and q for i-blocks >= 1 are
            # emitted inside i-block 0's loop, hidden under the ACT-bound
            # steady state.
            with (
                tc.tile_pool(name="ppB", bufs=2, space="PSUM") as ppB,
                tc.tile_pool(name="dsc", bufs=3, space="DRAM") as dsc,
            ):

                def emit_norm(avp, rsp, islp):
                    rcp = work.tile([128, 512], F32, tag="rcp")
                    # rows 1-63 are matmul-written zeros -> inf, never read
                    nc.vector.reciprocal(rcp[0:65, :], rsp[0:65, :])
                    # broadcast rcp rows over partitions via a DRAM bounce
                    # (SBUF DMA reads cannot have partition step 0; DRAM can)
                    scr = dsc.tile([2, 512], F32, tag="scr")
                    nc.sync.dma_start(scr, rcp[0:128:64, :])
                    rcpb = work.tile([128, 512], F32, tag="rcpb")
                    nc.sync.dma_start(
                        rcpb[0:64, :], scr[0:1, :].broadcast_to((64, 512))
                    )
                    nc.sync.dma_start(
                        rcpb[64:128, :], scr[1:2, :].broadcast_to((64, 512))
                    )
                    nc.vector.tensor_tensor(
                        a_fin[:, islp].bitcast(F32R), avp, rcpb, AL.mult
                    )

                def emit_proj(islp):
                    for ot in range(4):
                        pp = ppB.tile([128, 512], F32, tag="av")
                        mmr(
                            pp, pct_r[:, ot * 128 : (ot + 1) * 128],
                            a_fin[:, islp], start=True, stop=False,
                            skip_group_check=True,
                        )
                        mmr(
                            pp, pbr_r[:, ot * 128 : (ot + 1) * 128],
                            onesrow, start=False, stop=True,
                            skip_group_check=True,
                        )
                        o_sb = work.tile([128, 512], F32, tag="osb")
                        nc.vector.scalar_tensor_tensor(
                            o_sb, xs[ot][:, islp], flag_sb, pp,
                            op0=AL.mult, op1=AL.add,
                        )
                        nc.sync.dma_start(
                            out_d[ot * 128 : (ot + 1) * 128, islp], o_sb
                        )

                jt_norm, jt_proj = 2, min(12, NJ - 1)
                VT_AHEAD = 3
                prev = None
                for ib in range(NI):
                    isl = slice(ib * 512, (ib + 1) * 512)
                    av = ppB.tile([128, 512], F32, tag="av")
                    rs = ppB.tile([128, 512], F32, tag="rs")

                    def emit_avrs(jt, e, av=av, rs=rs):
                        # a += v @ e ; col-packed heads at out partitions 0/64;
                        # rowsums via ones-column matmuls (same PE mode)
                        first, last = jt == 0, jt == NJ - 1
                        nc.tensor.matmul(
                            av[0:64, :], vt_sb[:, jt * 128 : jt * 128 + 64],
                            e[:, 0:512], start=first, stop=last,
                            skip_group_check=True,
                        )
                        nc.tensor.matmul(
                            av[64:128, :], vt_sb[:, jt * 128 + 64 : jt * 128 + 128],
                            e[:, 512:1024], start=first, stop=last,
                            skip_group_check=True,
                        )
                        nc.tensor.matmul(
                            rs[0:64, :], onescol, e[:, 0:512],
                            start=first, stop=last, skip_group_check=True,
                        )
                        nc.tensor.matmul(
                            rs[64:128, :], onescol, e[:, 512:1024],
                            start=first, stop=last, skip_group_check=True,
                        )

                    if ib == 0:
                        for jt in range(min(VT_AHEAD, NJ)):
                            emit_vt(jt, ppB, "av")

                    pend = []  # (jt, e): av/rs emission lags two jt
                    for jt in range(NJ):
                        jsl = slice(jt * 128, (jt + 1) * 128)
                        st = ppB.tile([128, 1024], F32, tag="st")
                        # S^T[j, i] for both heads (row-packed, K=64 each)
                        nc.tensor.matmul(
                            st[:, 0:512], k_sb[0:64, jsl], q_sb[0:64, isl],
                            start=True, stop=True, skip_group_check=True,
                        )
                        nc.tensor.matmul(
                            st[:, 512:1024], k_sb[64:128, jsl], q_sb[64:128, isl],
                            start=True, stop=True, skip_group_check=True,
                        )
                        e = work.tile([128, 1024], BF16, tag="esb")
                        nc.scalar.activation(e, st, AF.Exp)
                        if ib == 0:
                            if jt + VT_AHEAD < NJ:
                                emit_vt(jt + VT_AHEAD, ppB, "av")
                            if jt >= 8 and (jt - 8) % 3 == 0 and (iq := (jt - 8) // 3 + 1) < NI:
                                emit_q(iq, ppB, "av")
                        pend.append((jt, e))
                        if len(pend) > 2:
                            emit_avrs(*pend.pop(0))
                        if prev is not None:
                            if jt == jt_norm:
                                emit_norm(*prev)
                            if jt == jt_proj:
                                emit_proj(prev[2])
                                prev = None
                    for p_ in pend:
                        emit_avrs(*p_)
                    prev = (av, rs, isl)
                emit_norm(*prev)
                emit_proj(prev[2])
    if hoist:
        _hoist_matmul_waits(nc)
    return nc


def make_in_maps(x, gn_weight, gn_bias, qkv_w, qkv_b, proj_w, proj_b, n):
    """Per-core input dicts (pure slicing / transposition / constant setup)."""
    bsel = np.kron(np.eye(8, dtype=np.float32), np.full((16, 16), 1.0 / 16.0, np.float32))
    zeros_pb = np.zeros((1, 512), np.float32)
    in_maps = []
    for core in range(NCORES):
        bi, p = divmod(core, 4)
        h0, h1 = 2 * p, 2 * p + 1
        rq = np.concatenate([qkv_w[192 * h : 192 * h + 64] for h in (h0, h1)])
        rk = np.concatenate([qkv_w[192 * h + 64 : 192 * h + 128] for h in (h0, h1)])
        rv = np.concatenate([qkv_w[192 * h + 128 : 192 * h + 192] for h in (h0, h1)])
        bq = np.concatenate([qkv_b[192 * h : 192 * h + 64] for h in (h0, h1)])
        bk = np.concatenate([qkv_b[192 * h + 64 : 192 * h + 128] for h in (h0, h1)])
        bv = np.concatenate([qkv_b[192 * h + 128 : 192 * h + 192] for h in (h0, h1)])
        flag = 1.0 if p == 0 else 0.0
        in_maps.append({
            "x": np.ascontiguousarray(x[bi].reshape(C, n)),
            "wq": np.ascontiguousarray(rq.T.reshape(4, 128, 128)),
            "wk": np.ascontiguousarray(rk.T.reshape(4, 128, 128)),
            "wv": np.ascontiguousarray(rv.T.reshape(4, 128, 128)),
            "qb": np.ascontiguousarray(bq.reshape(128, 1)),
            "kb": np.ascontiguousarray(bk.reshape(128, 1)),
            "vbb": np.ascontiguousarray(np.broadcast_to(bv[None, :], (128, 128))),
            "pct": np.ascontiguousarray(proj_w[:, 128 * p : 128 * (p + 1)].T),
            "pbr": (proj_b.reshape(1, 512).astype(np.float32) if flag else zeros_pb),
            "gnw": np.ascontiguousarray(gn_weight.reshape(4, 128, 1)),
            "gnb": np.ascontiguousarray(gn_bias.reshape(4, 128, 1)),
            "flag": np.full((128, 1), flag, np.float32),
            "bsel": bsel,
        })
    return in_maps


_NC_CACHE = {}


def kernel(x, gn_weight, gn_bias, qkv_w, qkv_b, proj_w, proj_b):
    b, c, H, W = x.shape
    n = H * W
    if n not in _NC_CACHE:
        _NC_CACHE[n] = build_nc(n)
    nc = _NC_CACHE[n]
    in_maps = make_in_maps(
        np.asarray(x, np.float32), np.asarray(gn_weight, np.float32),
        np.asarray(gn_bias, np.float32), np.asarray(qkv_w, np.float32),
        np.asarray(qkv_b, np.float32), np.asarray(proj_w, np.float32),
        np.asarray(proj_b, np.float32), n,
    )
    from concourse.bass_utils import run_bass_kernel_spmd

    res = run_bass_kernel_spmd(nc, in_maps, core_ids=list(range(NCORES)))
    parts = [r["out"] for r in res.results]
    out = np.empty((b, c, n), np.float32)
    for bi in range(b):
        acc = parts[4 * bi].astype(np.float32)
        for j in range(1, 4):
            acc = acc + parts[4 * bi + j]
        out[bi] = acc
    return out.reshape(b, c, H, W)
